# Initial kernel scaffold
#
"""Trainium2 Bass kernel for nn_ConditionedMambaTransition, 8 NeuronCores.

Strategy (batch-replicated tensor parallelism):
  - activations live feature-major [feature, batch] so BatchNorm batch-stats
    are free-dim reductions (bn_stats/bn_aggr), exact, no cross-core traffic
  - the MLP (mm1/mm2/mm3) + delta are computed full-batch on every core
  - the two big bilinear weights are sharded: Wb by d (32 rows/core),
    Wc by o (10 rows/core); per-batch einsum contractions are done as
    DVE product + PE matmul against constant 0/1 selector matrices
    accumulating in PSUM
  - one AllGather of the 32x1024 bf16 z_next shard per timestep is the
    only cross-core communication
  - all matmuls bf16 (weights pre-cast/pre-transposed/pre-blocked on the
    host), fp32 PSUM accumulation; biases folded into PSUM-evacuation
    activations; the recurrence state z is kept in fp32 on-chip
"""
import os

import numpy as np
import ml_dtypes

import concourse.bass as bass
import concourse.bacc as bacc
import concourse.tile as tile
import concourse.mybir as mybir
from concourse.bass_utils import run_bass_kernel_spmd

B, DD, DS, DU = 1024, 256, 256, 64
NOBS, HZ, H1, H2 = 80, 512, 512, 512
SIN = DD + DS + DU + 1  # 577
EPS = 1e-5
NC = 8
DSH, OSH = DD // NC, NOBS // NC  # 32, 10
NB_B, NB_C = DSH * DU // 128, OSH * DD // 128  # 16, 20
FH = 512  # free-dim half (batch 1024 = 2 halves)

BF16 = ml_dtypes.bfloat16
F32 = mybir.dt.float32
BF = mybir.dt.bfloat16
AF = mybir.ActivationFunctionType
ALU = mybir.AluOpType


def _blockify(wT, kchunks, ochunks):
    """[K, M] (K=kchunks*128 contraction, M=ochunks*128 out) ->
    [128, kchunks*ochunks*128] with block (kc, oc) at free offset
    (kc*ochunks + oc)*128."""
    K, M = wT.shape
    assert K == kchunks * 128 and M == ochunks * 128
    return np.ascontiguousarray(
        wT.reshape(kchunks, 128, ochunks, 128).transpose(1, 0, 2, 3)
        .reshape(128, kchunks * ochunks * 128))


def _prep(inputs, T):
    f32 = np.float32

    def g(name):
        return np.asarray(inputs[name], f32)

    z_dyn, z_static, dt, U = g("z_dyn"), g("z_static"), g("dt"), g("U")[:T]
    W1, W2, W3 = g("W1"), g("W2"), g("W3")
    g1, be1, g2, be2 = g("g1"), g("be1"), g("g2"), g("be2")
    b3 = g("b3")
    Wd, bd, Wb, bb, Wc, bc, D = g("Wd"), g("bd"), g("Wb"), g("bb"), g("Wc"), g("bc"), g("D")
    A = -np.exp(g("A_log"))

    bf = lambda a: np.ascontiguousarray(a).astype(BF16)

    # mm1 rhs chunk 4: [ut(64); dt(1); zeros(63)] per step
    Upack = np.zeros((T, 128, B), f32)
    Upack[:, :DU] = U.transpose(0, 2, 1)
    Upack[:, DU] = dt[:, 0][None, :]
    # ut*dt transposed, duplicated to 128 partitions (B-product operand)
    Udt = (U * dt[None]).transpose(0, 2, 1)
    Udtdup = np.concatenate([Udt, Udt], axis=1)

    W1T_pad = np.zeros((640, H1), f32)
    W1T_pad[:SIN] = W1.T

    selB = np.zeros((128, NB_B, DSH), f32)
    for oc in range(NB_B):
        selB[:64, oc, 2 * oc] = 1
        selB[64:, oc, 2 * oc + 1] = 1
    selC = np.zeros((128, OSH, OSH), f32)
    for o in range(OSH):
        selC[:, o, o] = 1

    col = lambda v, n: np.ascontiguousarray(v.reshape(n, 128).T)  # [128, n]

    shared = {
        "upk": bf(Upack),
        "udt": bf(Udtdup),
        "zst": bf(z_static.T),
        "z0b": bf(z_dyn.T),
        "w1": bf(_blockify(W1T_pad, 5, 4)),
        "w2": bf(_blockify(W2.T, 4, 4)),
        "w3": bf(_blockify(W3.T, 4, 4)),
        "selb": bf(selB.reshape(128, NB_B * DSH)),
        "selc": bf(selC.reshape(128, OSH * OSH)),
        "g1c": col(g1, 4), "be1c": col(be1, 4),
        "g2c": col(g2, 4), "be2c": col(be2, 4),
        "b3c": col(b3, 4),
    }

    in_maps = []
    for r in range(NC):
        d0, o0 = r * DSH, r * OSH
        WdT = Wd[d0:d0 + DSH].T  # [HZ, 32]
        wds = np.ascontiguousarray(
            WdT.reshape(4, 128, DSH).transpose(1, 0, 2).reshape(128, 4 * DSH))
        WbT = Wb[d0 * DU:(d0 + DSH) * DU].T  # [HZ, 2048]
        WcT = Wc[o0 * DD:(o0 + OSH) * DD].T  # [HZ, 2560]
        m = dict(shared)
        m.update({
            "wd": bf(wds),
            "wb": bf(_blockify(WbT, 4, NB_B)),
            "wc": bf(_blockify(WcT, 4, NB_C)),
            "dsl": bf(D[o0:o0 + OSH].T),                       # [64, 10]
            "bdc": np.ascontiguousarray(bd[d0:d0 + DSH].reshape(DSH, 1)),
            "ac": np.ascontiguousarray(A[d0:d0 + DSH].reshape(DSH, 1)),
            "bbc": col(bb[d0 * DU:(d0 + DSH) * DU], NB_B),
            "bcc": col(bc[o0 * DD:(o0 + OSH) * DD], NB_C),
            "z0f": np.ascontiguousarray(z_dyn.T[d0:d0 + DSH]),  # [32, B] f32
        })
        in_maps.append(m)
    return in_maps


def _build(nc, tc, io, T):
    sg = tc.tile_pool(name="sing", bufs=1).__enter__()
    wp = tc.tile_pool(name="work", bufs=2).__enter__()
    pp = tc.tile_pool(name="ps", bufs=1, space="PSUM").__enter__()
    dp = tc.tile_pool(name="dram", bufs=2, space="DRAM").__enter__()

    sync = nc.sync

    # ---- resident weights / constants ----
    def load(name, shape, dtype):
        t = sg.tile(shape, dtype, name=f"s_{name}")
        sync.dma_start(t, io[name][:])
        return t

    w1 = load("w1", [128, 5 * 4 * 128], BF)
    w2 = load("w2", [128, 4 * 4 * 128], BF)
    w3 = load("w3", [128, 4 * 4 * 128], BF)
    wd = load("wd", [128, 4 * DSH], BF)
    wb = load("wb", [128, 4 * NB_B * 128], BF)
    wc = load("wc", [128, 4 * NB_C * 128], BF)
    dsl = load("dsl", [64, OSH], BF)
    selb = load("selb", [128, NB_B * DSH], BF)
    selc = load("selc", [128, OSH * OSH], BF)
    g1c = load("g1c", [128, 4], F32)
    be1c = load("be1c", [128, 4], F32)
    g2c = load("g2c", [128, 4], F32)
    be2c = load("be2c", [128, 4], F32)
    b3c = load("b3c", [128, 4], F32)
    bdc = load("bdc", [DSH, 1], F32)
    ac = load("ac", [DSH, 1], F32)
    bbc = load("bbc", [128, NB_B], F32)
    bcc = load("bcc", [128, NB_C], F32)
    epsc = sg.tile([128, 1], F32, name="epsc")
    nc.vector.memset(epsc, EPS)

    zst = sg.tile([128, 2 * B // 2 * 2], BF, name="zst", padded_shape=[128, 2048])
    zst = zst[:, :2048]
    sync.dma_start(zst[:, 0:1024], io["zst"][0:128, :])
    sync.dma_start(zst[:, 1024:2048], io["zst"][128:256, :])

    z0b = sg.tile([128, 2048], BF, name="z0b")
    sync.dma_start(z0b[:, 0:1024], io["z0b"][0:128, :])
    sync.dma_start(z0b[:, 1024:2048], io["z0b"][128:256, :])
    z0f = sg.tile([DSH, B], F32, name="z0f")
    sync.dma_start(z0f, io["z0f"][:])

    rg = [list(range(NC))]

    def mlp_layer(wtile, xch, gt, bet, name, t):
        """one matmul [512x512-ish] + BN + relu -> bf16 [128, 4, 1024] tile"""
        hout = wp.tile([128, 4, 1024], BF, tag=f"h_{name}", name=f"h_{name}", bufs=2)
        nk = len(xch)
        for oc in range(4):
            st = wp.tile([128, 2, 6], F32, tag="st", name="st", bufs=4)
            pss = []
            for h in range(2):
                ps = pp.tile([128, FH], F32, tag="mlp", name="ps_mlp", bufs=3)
                for kc in range(nk):
                    nc.tensor.matmul(
                        ps, w_lhs(wtile, kc, oc, nk),
                        xch[kc][:, h * FH:(h + 1) * FH],
                        start=(kc == 0), stop=(kc == nk - 1))
                nc.vector.bn_stats(st[:, h, :], ps)
                pss.append(ps)
            mv = wp.tile([128, 2], F32, tag="mv", name="mv", bufs=4)
            nc.vector.bn_aggr(mv, st)
            a = wp.tile([128, 1], F32, tag="bna", name="bna", bufs=4)
            cb = wp.tile([128, 1], F32, tag="bnc", name="bnc", bufs=4)
            nc.scalar.activation(a, mv[:, 1:2], AF.Sqrt, bias=epsc, scale=1.0)
            nc.vector.reciprocal(a, a)
            nc.vector.tensor_mul(a, a, gt[:, oc:oc + 1])
            nc.vector.tensor_mul(cb, mv[:, 0:1], a)
            nc.vector.tensor_sub(cb, bet[:, oc:oc + 1], cb)
            for h in range(2):
                nc.scalar.activation(hout[:, oc, h * FH:(h + 1) * FH], pss[h],
                                     AF.Relu, bias=cb, scale=a)
        return hout

    def w_lhs(wtile, kc, oc, nk_tot):
        return wtile[:, (kc * 4 + oc) * 128:(kc * 4 + oc) * 128 + 128]

    zcur_b = z0b   # [128, 2048] bf16: z^T both chunks
    zcur_f = z0f   # [32, 1024] f32: own shard, master

    for t in range(T):
        with nc.named_scope(f"step{t}"):
            upkt = wp.tile([128, B], BF, tag="upk", name="upk", bufs=2)
            sync.dma_start(upkt, io["upk"][t])
            udtt = wp.tile([128, B], BF, tag="udt", name="udt", bufs=3)
            sync.dma_start(udtt, io["udt"][t])

            xch = [zcur_b[:, 0:1024], zcur_b[:, 1024:2048],
                   zst[:, 0:1024], zst[:, 1024:2048], upkt]
            h1 = mlp_layer(w1, xch, g1c, be1c, "h1", t)
            h1ch = [h1[:, kc, :] for kc in range(4)]
            h2 = mlp_layer(w2, h1ch, g2c, be2c, "h2", t)
            h2ch = [h2[:, kc, :] for kc in range(4)]

            # mm3 -> hz (bias b3 folded into evacuation)
            hz = wp.tile([128, 4, 1024], BF, tag="hz", name="hz", bufs=2)
            for oc in range(4):
                for h in range(2):
                    ps = pp.tile([128, FH], F32, tag="mlp", name="ps_mlp", bufs=3)
                    for kc in range(4):
                        nc.tensor.matmul(
                            ps, w_lhs(w3, kc, oc, 4),
                            h2ch[kc][:, h * FH:(h + 1) * FH],
                            start=(kc == 0), stop=(kc == 3))
                    nc.scalar.activation(hz[:, oc, h * FH:(h + 1) * FH], ps,
                                         AF.Identity, bias=b3c[:, oc:oc + 1],
                                         scale=1.0)
            hzch = [hz[:, kc, :] for kc in range(4)]

            # delta = softplus(hz @ Wd^T + bd), Abar = exp(delta * A)
            delta = wp.tile([DSH, B], F32, tag="delta", name="delta", bufs=2)
            for h in range(2):
                ps = pp.tile([DSH, FH], F32, tag="mlp", name="ps_mlp", bufs=3)
                for kc in range(4):
                    nc.tensor.matmul(ps, wd[:, kc * DSH:(kc + 1) * DSH],
                                     hzch[kc][:, h * FH:(h + 1) * FH],
                                     start=(kc == 0), stop=(kc == 3))
                nc.scalar.activation(delta[:, h * FH:(h + 1) * FH], ps,
                                     AF.Softplus, bias=bdc, scale=1.0)
            abar = wp.tile([DSH, B], F32, tag="abar", name="abar", bufs=2)
            nc.scalar.activation(abar, delta, AF.Exp, scale=ac)

            # ---- B part: zincr = sum_u (Wb hz + bb) * (u dt), sharded by d
            znf = wp.tile([DSH, B], F32, tag="znf", name="znf", bufs=3)
            for h in range(2):
                zi = pp.tile([DSH, FH], F32, tag="zincr", name="ps_zi", bufs=1)
                for oc in range(NB_B):
                    bp = pp.tile([128, FH], F32, tag="big", name="ps_big", bufs=3)
                    for kc in range(4):
                        nc.tensor.matmul(
                            bp, wb[:, (kc * NB_B + oc) * 128:(kc * NB_B + oc) * 128 + 128],
                            hzch[kc][:, h * FH:(h + 1) * FH],
                            start=(kc == 0), stop=(kc == 3))
                    prod = wp.tile([128, FH], BF, tag="bprod", name="bprod", bufs=3)
                    nc.vector.scalar_tensor_tensor(
                        prod, bp, bbc[:, oc:oc + 1], udtt[:, h * FH:(h + 1) * FH],
                        op0=ALU.add, op1=ALU.mult)
                    nc.tensor.matmul(zi, selb[:, oc * DSH:(oc + 1) * DSH], prod,
                                     start=(oc == 0), stop=(oc == NB_B - 1))
                # z_next = abar * z + delta * zincr  (fp32)
                t1 = wp.tile([DSH, FH], F32, tag="zt1", name="zt1", bufs=2)
                nc.vector.tensor_mul(t1, abar[:, h * FH:(h + 1) * FH],
                                     zcur_f[:, h * FH:(h + 1) * FH])
                t2 = wp.tile([DSH, FH], F32, tag="zt2", name="zt2", bufs=2)
                nc.vector.tensor_mul(t2, delta[:, h * FH:(h + 1) * FH], zi)
                nc.vector.tensor_add(znf[:, h * FH:(h + 1) * FH], t1, t2)

            sync.dma_start(io["zs"][t], znf)
            znb = wp.tile([DSH, B], BF, tag="znb", name="znb", bufs=2)
            nc.vector.tensor_copy(znb, znf)

            agi = dp.tile([DSH, B], BF, tag="agi", name="agi", bufs=2)
            sync.dma_start(agi, znb)
            ago = dp.tile([DD, B], BF, tag="ago", name="ago", bufs=2,
                          addr_space="Shared")
            nc.gpsimd.collective_compute(
                "AllGather", ALU.bypass, replica_groups=rg,
                ins=[agi.opt()], outs=[ago.opt()])
            znxt_b = wp.tile([128, 2048], BF, tag="ztb", name="ztb", bufs=3)
            sync.dma_start(znxt_b[:, 0:1024], ago[0:128, :])
            sync.dma_start(znxt_b[:, 1024:2048], ago[128:256, :])

            # ---- C part: yt = sum_d (Wc hz + bc) * z_next + D (u dt), by o
            for h in range(2):
                yt = pp.tile([OSH, FH], F32, tag="yt", name="ps_yt", bufs=1)
                nc.tensor.matmul(yt, dsl, udtt[0:64, h * FH:(h + 1) * FH],
                                 start=True, stop=False)
                for oc in range(NB_C):
                    cp = pp.tile([128, FH], F32, tag="big", name="ps_big", bufs=3)
                    for kc in range(4):
                        nc.tensor.matmul(
                            cp, wc[:, (kc * NB_C + oc) * 128:(kc * NB_C + oc) * 128 + 128],
                            hzch[kc][:, h * FH:(h + 1) * FH],
                            start=(kc == 0), stop=(kc == 3))
                    cts = wp.tile([128, FH], BF, tag="cts", name="cts", bufs=3)
                    nc.scalar.activation(cts, cp, AF.Identity,
                                         bias=bcc[:, oc:oc + 1], scale=1.0)
                    cprod = wp.tile([128, FH], BF, tag="cprod", name="cprod", bufs=3)
                    nc.vector.tensor_mul(
                        cprod, cts,
                        znxt_b[:, (oc % 2) * 1024 + h * FH:(oc % 2) * 1024 + h * FH + FH])
                    nc.tensor.matmul(yt, selc[:, (oc // 2) * OSH:(oc // 2 + 1) * OSH],
                                     cprod, start=False, stop=(oc == NB_C - 1))
                yts = wp.tile([OSH, FH], F32, tag="yts", name="yts", bufs=2)
                nc.scalar.activation(yts, yt, AF.Copy)
                sync.dma_start(io["ys"][t, :, h * FH:(h + 1) * FH], yts)

            zcur_b = znxt_b
            zcur_f = znf


def build_program(T):
    nc = bacc.Bacc("TRN2", target_bir_lowering=False, debug=False,
                   enable_asserts=False, num_devices=NC)
    io = {}
    io["upk"] = nc.dram_tensor("upk", [T, 128, B], BF, kind="ExternalInput").ap()
    io["udt"] = nc.dram_tensor("udt", [T, 128, B], BF, kind="ExternalInput").ap()
    for name, shape, dt_ in [
        ("zst", [DD, B], BF), ("z0b", [DD, B], BF), ("z0f", [DSH, B], F32),
        ("w1", [128, 5 * 4 * 128], BF), ("w2", [128, 16 * 128], BF),
        ("w3", [128, 16 * 128], BF), ("wd", [128, 4 * DSH], BF),
        ("wb", [128, 4 * NB_B * 128], BF), ("wc", [128, 4 * NB_C * 128], BF),
        ("dsl", [64, OSH], BF), ("selb", [128, NB_B * DSH], BF),
        ("selc", [128, OSH * OSH], BF),
        ("g1c", [128, 4], F32), ("be1c", [128, 4], F32),
        ("g2c", [128, 4], F32), ("be2c", [128, 4], F32), ("b3c", [128, 4], F32),
        ("bdc", [DSH, 1], F32), ("ac", [DSH, 1], F32),
        ("bbc", [128, NB_B], F32), ("bcc", [128, NB_C], F32),
    ]:
        io[name] = nc.dram_tensor(name, shape, dt_, kind="ExternalInput").ap()
    io["zs"] = nc.dram_tensor("zs", [T, DSH, B], F32, kind="ExternalOutput").ap()
    io["ys"] = nc.dram_tensor("ys", [T, OSH, B], F32, kind="ExternalOutput").ap()

    with tile.TileContext(nc) as tc:
        _build(nc, tc, io, T)
    nc.compile()
    return nc


def kernel(**inputs):
    T = int(os.environ.get("KERNEL_T", "64"))
    in_maps = _prep(inputs, T)
    nc = build_program(T)
    trace = os.environ.get("KERNEL_TRACE", "0") == "1"
    res = run_bass_kernel_spmd(nc, in_maps, core_ids=list(range(NC)), trace=trace)
    Zs = np.concatenate([res.results[r]["zs"] for r in range(NC)], axis=1)
    Ys = np.concatenate([res.results[r]["ys"] for r in range(NC)], axis=1)
    if trace and res.exec_time_ns is not None:
        print(f"HW exec time: {res.exec_time_ns} ns")
    return Zs.transpose(0, 2, 1).astype(np.float32), Ys.transpose(0, 2, 1).astype(np.float32)


if __name__ == "__main__":
    # smoke-build only
    t0 = __import__("time").time()
    nc = build_program(int(os.environ.get("KERNEL_T", "2")))
    print("built+compiled in", __import__("time").time() - t0, "s")


# revision 8
# speedup vs baseline: 1.2004x; 1.2004x over previous
"""Trainium2 Bass kernel for nn_ConditionedMambaTransition, 8 NeuronCores.

Strategy (batch-replicated tensor parallelism):
  - activations live feature-major [feature, batch] so BatchNorm batch-stats
    are free-dim reductions (bn_stats/bn_aggr), exact, no cross-core traffic
  - the MLP (mm1/mm2/mm3) + delta are computed full-batch on every core
  - the two big bilinear weights are sharded: Wb by d (32 rows/core),
    Wc by o (10 rows/core); per-batch einsum contractions are done as
    DVE product + PE matmul against constant 0/1 selector matrices
    accumulating in PSUM
  - one AllGather of the 32x1024 bf16 z_next shard per timestep is the
    only cross-core communication
  - all matmuls bf16 (weights pre-cast/pre-transposed/pre-blocked on the
    host), fp32 PSUM accumulation; biases folded into PSUM-evacuation
    activations; the recurrence state z is kept in fp32 on-chip
"""
import os

import numpy as np
import ml_dtypes

import concourse.bass as bass
import concourse.bacc as bacc
import concourse.tile as tile
import concourse.mybir as mybir
from concourse.bass_utils import run_bass_kernel_spmd

B, DD, DS, DU = 1024, 256, 256, 64
NOBS, HZ, H1, H2 = 80, 512, 512, 512
SIN = DD + DS + DU + 1  # 577
EPS = 1e-5
NC = 8
DSH, OSH = DD // NC, NOBS // NC  # 32, 10
NB_B, NB_C = DSH * DU // 128, OSH * DD // 128  # 16, 20
FH = 512  # free-dim half (batch 1024 = 2 halves)

BF16 = ml_dtypes.bfloat16
F32 = mybir.dt.float32
BF = mybir.dt.bfloat16
AF = mybir.ActivationFunctionType
ALU = mybir.AluOpType


def _blockify(wT, kchunks, ochunks):
    """[K, M] (K=kchunks*128 contraction, M=ochunks*128 out) ->
    [128, kchunks*ochunks*128] with block (kc, oc) at free offset
    (kc*ochunks + oc)*128."""
    K, M = wT.shape
    assert K == kchunks * 128 and M == ochunks * 128
    return np.ascontiguousarray(
        wT.reshape(kchunks, 128, ochunks, 128).transpose(1, 0, 2, 3)
        .reshape(128, kchunks * ochunks * 128))


def _prep(inputs, T):
    f32 = np.float32

    def g(name):
        return np.asarray(inputs[name], f32)

    z_dyn, z_static, dt, U = g("z_dyn"), g("z_static"), g("dt"), g("U")[:T]
    W1, W2, W3 = g("W1"), g("W2"), g("W3")
    g1, be1, g2, be2 = g("g1"), g("be1"), g("g2"), g("be2")
    b3 = g("b3")
    Wd, bd, Wb, bb, Wc, bc, D = g("Wd"), g("bd"), g("Wb"), g("bb"), g("Wc"), g("bc"), g("D")
    A = -np.exp(g("A_log"))

    bf = lambda a: np.ascontiguousarray(a).astype(BF16)

    # mm1 rhs chunk 4: [ut(64); dt(1); zeros(63)] per step
    Upack = np.zeros((T, 128, B), f32)
    Upack[:, :DU] = U.transpose(0, 2, 1)
    Upack[:, DU] = dt[:, 0][None, :]
    # ut*dt transposed, duplicated to 128 partitions (B-product operand)
    Udt = (U * dt[None]).transpose(0, 2, 1)
    Udtdup = np.concatenate([Udt, Udt], axis=1)

    W1T_pad = np.zeros((640, H1), f32)
    W1T_pad[:SIN] = W1.T

    selB = np.zeros((128, NB_B, DSH), f32)
    for oc in range(NB_B):
        selB[:64, oc, 2 * oc] = 1
        selB[64:, oc, 2 * oc + 1] = 1
    selC = np.zeros((128, OSH, OSH), f32)
    for o in range(OSH):
        selC[:, o, o] = 1

    col = lambda v, n: np.ascontiguousarray(v.reshape(n, 128).T)  # [128, n]

    shared = {
        "upk": bf(Upack),
        "udt": bf(Udtdup),
        "zst": bf(z_static.T),
        "z0b": bf(z_dyn.T),
        "w1": bf(_blockify(W1T_pad, 5, 4)),
        "w2": bf(_blockify(W2.T, 4, 4)),
        "w3": bf(_blockify(W3.T, 4, 4)),
        "selb": bf(selB.reshape(128, NB_B * DSH)),
        "selc": bf(selC.reshape(128, OSH * OSH)),
        "g1c": col(g1, 4), "be1c": col(be1, 4),
        "g2c": col(g2, 4), "be2c": col(be2, 4),
        "b3c": col(b3, 4),
    }

    in_maps = []
    for r in range(NC):
        d0, o0 = r * DSH, r * OSH
        WdT = Wd[d0:d0 + DSH].T  # [HZ, 32]
        wds = np.ascontiguousarray(
            WdT.reshape(4, 128, DSH).transpose(1, 0, 2).reshape(128, 4 * DSH))
        WbT = Wb[d0 * DU:(d0 + DSH) * DU].T  # [HZ, 2048]
        WcT = Wc[o0 * DD:(o0 + OSH) * DD].T  # [HZ, 2560]
        m = dict(shared)
        m.update({
            "wd": bf(wds),
            "wb": bf(_blockify(WbT, 4, NB_B)),
            "wc": bf(_blockify(WcT, 4, NB_C)),
            "dsl": bf(D[o0:o0 + OSH].T),                       # [64, 10]
            "bdc": np.ascontiguousarray(bd[d0:d0 + DSH].reshape(DSH, 1)),
            "ac": np.ascontiguousarray(A[d0:d0 + DSH].reshape(DSH, 1)),
            "bbc": col(bb[d0 * DU:(d0 + DSH) * DU], NB_B),
            "bcc": col(bc[o0 * DD:(o0 + OSH) * DD], NB_C),
            "z0f": np.ascontiguousarray(z_dyn.T[d0:d0 + DSH]),  # [32, B] f32
        })
        in_maps.append(m)
    return in_maps


def _build(nc, tc, io, T, ctx):
    sg = ctx.enter_context(tc.tile_pool(name="sing", bufs=1))
    wp = ctx.enter_context(tc.tile_pool(name="work", bufs=2))
    pp = ctx.enter_context(tc.tile_pool(name="ps", bufs=1, space="PSUM"))
    dp = ctx.enter_context(tc.tile_pool(name="dram", bufs=2, space="DRAM"))

    sync = nc.sync

    # ---- resident weights / constants ----
    def load(name, shape, dtype):
        t = sg.tile(shape, dtype, name=f"s_{name}")
        sync.dma_start(t, io[name][:])
        return t

    w1 = load("w1", [128, 5 * 4 * 128], BF)
    w2 = load("w2", [128, 4 * 4 * 128], BF)
    w3 = load("w3", [128, 4 * 4 * 128], BF)
    wd = load("wd", [128, 4 * DSH], BF)
    wb = load("wb", [128, 4 * NB_B * 128], BF)
    wc = load("wc", [128, 4 * NB_C * 128], BF)
    dsl = load("dsl", [64, OSH], BF)
    selb = load("selb", [128, NB_B * DSH], BF)
    selc = load("selc", [128, OSH * OSH], BF)
    g1c = load("g1c", [128, 4], F32)
    be1c = load("be1c", [128, 4], F32)
    g2c = load("g2c", [128, 4], F32)
    be2c = load("be2c", [128, 4], F32)
    b3c = load("b3c", [128, 4], F32)
    bdc = load("bdc", [DSH, 1], F32)
    ac = load("ac", [DSH, 1], F32)
    bbc = load("bbc", [128, NB_B], F32)
    bcc = load("bcc", [128, NB_C], F32)
    epsc = sg.tile([128, 1], F32, name="epsc")
    nc.vector.memset(epsc, EPS)

    zst = sg.tile([128, 2048], BF, name="zst")
    sync.dma_start(zst[:, 0:1024], io["zst"][0:128, :])
    sync.dma_start(zst[:, 1024:2048], io["zst"][128:256, :])

    z0b = sg.tile([128, 2048], BF, name="z0b")
    sync.dma_start(z0b[:, 0:1024], io["z0b"][0:128, :])
    sync.dma_start(z0b[:, 1024:2048], io["z0b"][128:256, :])
    z0f = sg.tile([DSH, B], F32, name="z0f")
    sync.dma_start(z0f, io["z0f"][:])

    rg = [list(range(NC))]

    def mlp_layer(wtile, xch, gt, bet, name, t):
        """one matmul [512x512-ish] + BN + relu -> bf16 [128, 4, 1024] tile"""
        hout = wp.tile([128, 4, 1024], BF, tag=f"h_{name}", name=f"h_{name}", bufs=1)
        nk = len(xch)
        for oc in range(4):
            st = wp.tile([128, 2, 6], F32, tag="st", name="st", bufs=4)
            pss = []
            for h in range(2):
                ps = pp.tile([128, FH], F32, tag="mlp", name="ps_mlp", bufs=3)
                for kc in range(nk):
                    nc.tensor.matmul(
                        ps, w_lhs(wtile, kc, oc, nk),
                        xch[kc][:, h * FH:(h + 1) * FH],
                        start=(kc == 0), stop=(kc == nk - 1))
                nc.vector.bn_stats(st[:, h, :], ps)
                pss.append(ps)
            mv = wp.tile([128, 2], F32, tag="mv", name="mv", bufs=4)
            nc.vector.bn_aggr(mv, st)
            a = wp.tile([128, 1], F32, tag="bna", name="bna", bufs=4)
            cb = wp.tile([128, 1], F32, tag="bnc", name="bnc", bufs=4)
            nc.scalar.activation(a, mv[:, 1:2], AF.Sqrt, bias=epsc, scale=1.0)
            nc.vector.reciprocal(a, a)
            nc.vector.tensor_mul(a, a, gt[:, oc:oc + 1])
            nc.vector.tensor_mul(cb, mv[:, 0:1], a)
            nc.vector.tensor_sub(cb, bet[:, oc:oc + 1], cb)
            for h in range(2):
                nc.scalar.activation(hout[:, oc, h * FH:(h + 1) * FH], pss[h],
                                     AF.Relu, bias=cb, scale=a)
        return hout

    def w_lhs(wtile, kc, oc, nk_tot):
        return wtile[:, (kc * 4 + oc) * 128:(kc * 4 + oc) * 128 + 128]

    zcur_b = z0b   # [128, 2048] bf16: z^T both chunks
    zcur_f = z0f   # [32, 1024] f32: own shard, master

    for t in range(T):
        with nc.named_scope(f"step{t}"):
            upkt = wp.tile([128, B], BF, tag="upk", name="upk", bufs=2)
            sync.dma_start(upkt, io["upk"][t])
            udtt = wp.tile([128, B], BF, tag="udt", name="udt", bufs=3)
            sync.dma_start(udtt, io["udt"][t])

            xch = [zcur_b[:, 0:1024], zcur_b[:, 1024:2048],
                   zst[:, 0:1024], zst[:, 1024:2048], upkt]
            h1 = mlp_layer(w1, xch, g1c, be1c, "h1", t)
            h1ch = [h1[:, kc, :] for kc in range(4)]
            h2 = mlp_layer(w2, h1ch, g2c, be2c, "h2", t)
            h2ch = [h2[:, kc, :] for kc in range(4)]

            # mm3 -> hz (bias b3 folded into evacuation)
            hz = wp.tile([128, 4, 1024], BF, tag="hz", name="hz", bufs=2)
            for oc in range(4):
                for h in range(2):
                    ps = pp.tile([128, FH], F32, tag="mlp", name="ps_mlp", bufs=3)
                    for kc in range(4):
                        nc.tensor.matmul(
                            ps, w_lhs(w3, kc, oc, 4),
                            h2ch[kc][:, h * FH:(h + 1) * FH],
                            start=(kc == 0), stop=(kc == 3))
                    nc.scalar.activation(hz[:, oc, h * FH:(h + 1) * FH], ps,
                                         AF.Identity, bias=b3c[:, oc:oc + 1],
                                         scale=1.0)
            hzch = [hz[:, kc, :] for kc in range(4)]

            # delta = softplus(hz @ Wd^T + bd) = relu(x) + ln1p(e^-|x|),
            # ln1p via cubic init + 1 Newton step (no Softplus/Ln act table).
            SPC = (-0.07689484303505646, 0.25934442297466137,
                   -0.4894089294334617, 0.9999470683861231)
            delta = wp.tile([DSH, B], F32, tag="delta", name="delta", bufs=2)
            for h in range(2):
                hs = slice(h * FH, (h + 1) * FH)
                ps = pp.tile([DSH, FH], F32, tag="mlp", name="ps_mlp", bufs=3)
                for kc in range(4):
                    nc.tensor.matmul(ps, wd[:, kc * DSH:(kc + 1) * DSH],
                                     hzch[kc][:, h * FH:(h + 1) * FH],
                                     start=(kc == 0), stop=(kc == 3))
                ex = wp.tile([DSH, FH], F32, tag="sp_ex", name="sp_ex", bufs=1)
                nc.scalar.activation(ex, ps, AF.Exp, bias=bdc, scale=1.0)
                rx = wp.tile([DSH, FH], F32, tag="sp_rx", name="sp_rx", bufs=1)
                nc.vector.reciprocal(rx, ex)
                uu = wp.tile([DSH, FH], F32, tag="sp_u", name="sp_u", bufs=1)
                nc.vector.tensor_tensor(uu, ex, rx, op=ALU.min)  # e^-|x|
                rl = wp.tile([DSH, FH], F32, tag="sp_rl", name="sp_rl", bufs=1)
                nc.scalar.activation(rl, ps, AF.Relu, bias=bdc, scale=1.0)
                # u*(c0 + u*(c1 + u*(c2 + u*c3))) via h <- (h + k)*u
                hh = wp.tile([DSH, FH], F32, tag="sp_h", name="sp_h", bufs=1)
                nc.vector.tensor_scalar(hh, uu, SPC[0], 0.0,
                                        op0=ALU.mult, op1=ALU.add)
                for cc in (SPC[1], SPC[2], SPC[3]):
                    nc.vector.scalar_tensor_tensor(hh, hh, cc, uu,
                                                   op0=ALU.add, op1=ALU.mult)
                # hh = u * poly(u) ~= ln1p(u); newton: s = hh-1 + (1+u)e^-hh
                en = wp.tile([DSH, FH], F32, tag="sp_en", name="sp_en", bufs=1)
                nc.scalar.activation(en, hh, AF.Exp, scale=-1.0)
                q = wp.tile([DSH, FH], F32, tag="sp_q", name="sp_q", bufs=1)
                nc.vector.scalar_tensor_tensor(q, uu, 1.0, en,
                                               op0=ALU.add, op1=ALU.mult)
                nc.vector.scalar_tensor_tensor(hh, hh, -1.0, q,
                                               op0=ALU.add, op1=ALU.add)
                nc.vector.tensor_add(delta[:, hs], rl, hh)
            abar = wp.tile([DSH, B], F32, tag="abar", name="abar", bufs=2)
            nc.scalar.activation(abar, delta, AF.Exp, scale=ac)

            # ---- B part: zincr = sum_u (Wb hz + bb) * (u dt), sharded by d
            NOB = os.environ.get("KERNEL_NOBC", "0") == "1"
            znf = wp.tile([DSH, B], F32, tag="znf", name="znf", bufs=3)
            for h in range(2):
                zi = pp.tile([DSH, FH], F32, tag="zincr", name="ps_zi", bufs=1)
                for oc in range(1 if NOB else NB_B):
                    bp = pp.tile([128, FH], F32, tag="big", name="ps_big", bufs=3)
                    for kc in range(4):
                        nc.tensor.matmul(
                            bp, wb[:, (kc * NB_B + oc) * 128:(kc * NB_B + oc) * 128 + 128],
                            hzch[kc][:, h * FH:(h + 1) * FH],
                            start=(kc == 0), stop=(kc == 3))
                    prod = wp.tile([128, FH], BF, tag="bprod", name="bprod", bufs=3)
                    nc.vector.scalar_tensor_tensor(
                        prod, bp, bbc[:, oc:oc + 1], udtt[:, h * FH:(h + 1) * FH],
                        op0=ALU.add, op1=ALU.mult)
                    nc.tensor.matmul(zi, selb[:, oc * DSH:(oc + 1) * DSH], prod,
                                     start=(oc == 0),
                                     stop=(oc == (0 if NOB else NB_B - 1)))
                # z_next = abar * z + delta * zincr  (fp32)
                t1 = wp.tile([DSH, FH], F32, tag="zt1", name="zt1", bufs=2)
                nc.vector.tensor_mul(t1, abar[:, h * FH:(h + 1) * FH],
                                     zcur_f[:, h * FH:(h + 1) * FH])
                t2 = wp.tile([DSH, FH], F32, tag="zt2", name="zt2", bufs=2)
                nc.vector.tensor_mul(t2, delta[:, h * FH:(h + 1) * FH], zi)
                nc.vector.tensor_add(znf[:, h * FH:(h + 1) * FH], t1, t2)

            sync.dma_start(io["zs"][t], znf)
            znb = wp.tile([DSH, B], BF, tag="znb", name="znb", bufs=2)
            nc.vector.tensor_copy(znb, znf)

            agi = dp.tile([DSH, B], BF, tag="agi", name="agi", bufs=2)
            sync.dma_start(agi, znb)
            ago = dp.tile([DD, B], BF, tag="ago", name="ago", bufs=2,
                          addr_space="Shared")
            if os.environ.get("KERNEL_NOAG", "0") == "1":
                sync.dma_start(ago[0:DSH, :], agi)
            else:
                nc.gpsimd.collective_compute(
                    "AllGather", ALU.bypass, replica_groups=rg,
                    ins=[agi.opt()], outs=[ago.opt()])
            znxt_b = wp.tile([128, 2048], BF, tag="ztb", name="ztb", bufs=3)
            sync.dma_start(znxt_b[:, 0:1024], ago[0:128, :])
            sync.dma_start(znxt_b[:, 1024:2048], ago[128:256, :])

            # ---- C part: yt = sum_d (Wc hz + bc) * z_next + D (u dt), by o
            for h in range(2):
                yt = pp.tile([OSH, FH], F32, tag="yt", name="ps_yt", bufs=1)
                nc.tensor.matmul(yt, dsl, udtt[0:64, h * FH:(h + 1) * FH],
                                 start=True, stop=False)
                for oc in range(1 if NOB else NB_C):
                    cp = pp.tile([128, FH], F32, tag="big", name="ps_big", bufs=3)
                    for kc in range(4):
                        nc.tensor.matmul(
                            cp, wc[:, (kc * NB_C + oc) * 128:(kc * NB_C + oc) * 128 + 128],
                            hzch[kc][:, h * FH:(h + 1) * FH],
                            start=(kc == 0), stop=(kc == 3))
                    cts = wp.tile([128, FH], BF, tag="cts", name="cts", bufs=3)
                    nc.scalar.activation(cts, cp, AF.Identity,
                                         bias=bcc[:, oc:oc + 1], scale=1.0)
                    cprod = wp.tile([128, FH], BF, tag="cprod", name="cprod", bufs=3)
                    nc.vector.tensor_mul(
                        cprod, cts,
                        znxt_b[:, (oc % 2) * 1024 + h * FH:(oc % 2) * 1024 + h * FH + FH])
                    nc.tensor.matmul(yt, selc[:, (oc // 2) * OSH:(oc // 2 + 1) * OSH],
                                     cprod, start=False,
                                     stop=(oc == (0 if NOB else NB_C - 1)))
                yts = wp.tile([OSH, FH], F32, tag="yts", name="yts", bufs=2)
                nc.scalar.activation(yts, yt, AF.Copy)
                sync.dma_start(io["ys"][t, :, h * FH:(h + 1) * FH], yts)

            zcur_b = znxt_b
            zcur_f = znf


def build_program(T):
    nc = bacc.Bacc("TRN2", target_bir_lowering=False, debug=False,
                   enable_asserts=False, num_devices=NC)
    io = {}
    io["upk"] = nc.dram_tensor("upk", [T, 128, B], BF, kind="ExternalInput").ap()
    io["udt"] = nc.dram_tensor("udt", [T, 128, B], BF, kind="ExternalInput").ap()
    for name, shape, dt_ in [
        ("zst", [DD, B], BF), ("z0b", [DD, B], BF), ("z0f", [DSH, B], F32),
        ("w1", [128, 5 * 4 * 128], BF), ("w2", [128, 16 * 128], BF),
        ("w3", [128, 16 * 128], BF), ("wd", [128, 4 * DSH], BF),
        ("wb", [128, 4 * NB_B * 128], BF), ("wc", [128, 4 * NB_C * 128], BF),
        ("dsl", [64, OSH], BF), ("selb", [128, NB_B * DSH], BF),
        ("selc", [128, OSH * OSH], BF),
        ("g1c", [128, 4], F32), ("be1c", [128, 4], F32),
        ("g2c", [128, 4], F32), ("be2c", [128, 4], F32), ("b3c", [128, 4], F32),
        ("bdc", [DSH, 1], F32), ("ac", [DSH, 1], F32),
        ("bbc", [128, NB_B], F32), ("bcc", [128, NB_C], F32),
    ]:
        io[name] = nc.dram_tensor(name, shape, dt_, kind="ExternalInput").ap()
    io["zs"] = nc.dram_tensor("zs", [T, DSH, B], F32, kind="ExternalOutput").ap()
    io["ys"] = nc.dram_tensor("ys", [T, OSH, B], F32, kind="ExternalOutput").ap()

    from contextlib import ExitStack
    with tile.TileContext(nc) as tc, ExitStack() as ctx:
        _build(nc, tc, io, T, ctx)
    nc.compile()
    return nc


def kernel(**inputs):
    T = int(os.environ.get("KERNEL_T", "64"))
    in_maps = _prep(inputs, T)
    nc = build_program(T)
    trace = os.environ.get("KERNEL_TRACE", "0") == "1"
    res = run_bass_kernel_spmd(nc, in_maps, core_ids=list(range(NC)), trace=trace)
    Zs = np.concatenate([res.results[r]["zs"] for r in range(NC)], axis=1)
    Ys = np.concatenate([res.results[r]["ys"] for r in range(NC)], axis=1)
    if trace and res.exec_time_ns is not None:
        print(f"HW exec time: {res.exec_time_ns} ns")
    return Zs.transpose(0, 2, 1).astype(np.float32), Ys.transpose(0, 2, 1).astype(np.float32)


if __name__ == "__main__":
    # smoke-build only
    t0 = __import__("time").time()
    nc = build_program(int(os.environ.get("KERNEL_T", "2")))
    print("built+compiled in", __import__("time").time() - t0, "s")
